# revision 14
# baseline (speedup 1.0000x reference)
"""Trainium2 Bass kernel for the AttnBlock-style attention module.

Reference computation (note softmax over axis=1, the *i* axis):
    q = wq @ x + bq ; k = wk @ x + bk ; v = wv @ x + bv      (per-pixel 1x1 conv)
    s[b,i,j] = (q[b,:,i] . k[b,:,j]) * C**-0.5
    attn = softmax_i(s)                                      (normalize over i!)
    out[b,c,i] = sum_j attn[b,i,j] v[b,c,j]
    y = wp @ out + bp

Math folding: softmax over i is invariant to per-j constants, so
    s_ij = q_i.k_j = x_i^T (A x_j + c1) + (const_j)    with A = wq^T wk,
                                                       c1 = wq^T bk.
The q projection disappears entirely: the moving side of the score GEMM is
raw x (already in SBUF from the input DMA), and only u = A x + c1 is
projected for the j-columns. Likewise wp is folded into v (w2 = wp@wv).

Sharding: 8 cores = 4 batches x 2 j-halves. The softmax over i is local to a
j-split (it normalizes each attention *column* j over all i). Each core gets x
with its j-half rotated to columns 0..2047 (a pure permutation of the pixel
axis; the host un-rotates the partial output). Each core:
  - computes u (j-half) and v (j-half) with bf16 GEMMs,
  - s_T[j, i] = u^T x  (j on partitions -> softmax reduction is free-axis),
  - attn = exp(s/16) stored unnormalized in bf16; per-j denominators D[j]
    via the fused activation accum_out; 1/D folded into v rows,
  - out_partial[c, i] = sum_{j in half} v_scaled[c,j] attn_T[j,i],
  - y_partial accumulated in f32 SBUF (bias bp added on host).
Host un-rotates and sums the two j-half partials per batch and adds bp.

Out-accumulation is staged in five j-groups (6,4,4,1,1): the chains for a
group are interleaved into later jts' score/exp stream so the PE fills the
ACT-bound stretches, and the tiny trailing groups shrink the post-last-exp
tail to ~4 matmuls per (iq,ch) slice.
"""

import numpy as np

import concourse.bass as bass
import concourse.mybir as mybir
import concourse.tile as tile
from concourse import bacc
from concourse import bass_utils

P = 128
B = 4
C = 256
N = 4096          # 64*64 pixels
NJ = 2048         # j columns per core
NJT = NJ // P     # 16 j tiles
SCALE = 1.0 / np.sqrt(C).item()   # 1/16

F32 = mybir.dt.float32
BF16 = mybir.dt.bfloat16
AF = mybir.ActivationFunctionType

# j-tile boundaries of the five out-accumulation groups
GRP = [(0, 6), (6, 10), (10, 14), (14, 15), (15, 16)]


def _build_module():
    nc = bacc.Bacc("TRN2", target_bir_lowering=False, debug=False, num_devices=8)

    x_t = nc.dram_tensor("x", [C, N], BF16, kind="ExternalInput")
    w_t = nc.dram_tensor("wT", [2, C, C], BF16, kind="ExternalInput")  # A.T, (wp@wv).T
    b_t = nc.dram_tensor("b", [1, C], F32, kind="ExternalInput")      # c1 = wq.T @ bk
    bv_t = nc.dram_tensor("bv", [1, C], F32, kind="ExternalInput")    # wp@bv
    y_t = nc.dram_tensor("y", [C, N], F32, kind="ExternalOutput")

    with tile.TileContext(nc) as tc:
        _emit(nc, tc, x_t, w_t, b_t, bv_t, y_t)
    nc.compile()
    return nc


def _emit(nc, tc, x_t, w_t, b_t, bv_t, y_t):
    from contextlib import ExitStack

    with ExitStack() as top:
        const = top.enter_context(tc.tile_pool(name="const", bufs=1))
        big = top.enter_context(tc.tile_pool(name="big", bufs=1))

        # ---- u's weight first on the sync queue, then the x blocks ------
        # w_all[:, 2*w + ci, :] = rows ci*128.. of weight w's transpose
        # slots 0,1: A.T chunks; 2,3: w2.T chunks; 4,5: bv as f32 bits
        w_all = const.tile([P, 6, C], BF16, tag="w_all", name="w_all")
        nc.sync.dma_start(
            w_all[:, 0:2, :].rearrange("p w f -> p (w f)").rearrange(
                "p (c f) -> p c f", c=2),
            bass.AP(tensor=w_t, offset=0, ap=[[C, P], [P * C, 2], [1, C]]),
        )

        xp = top.enter_context(tc.tile_pool(name="xload", bufs=1))
        XBLK = [(0, 512), (512, 512), (1024, 1024), (2048, 1024), (3072, 1024)]
        xb = [xp.tile([P, 2, w], BF16, tag=f"xb{b}", name=f"xb{b}")
              for b, (lo, w) in enumerate(XBLK)]
        for b, (lo, w) in enumerate(XBLK):
            nc.sync.dma_start(
                xb[b][:],
                bass.AP(tensor=x_t, offset=lo,
                        ap=[[N, P], [P * N, 2], [1, w]]),
            )

        def xsl(ci, lo, size):
            # x[ci*128:(ci+1)*128, lo:lo+size] as an AP (within one block)
            for b, (blo, w) in enumerate(XBLK):
                if blo <= lo and lo + size <= blo + w:
                    return xb[b][:, ci, lo - blo:lo - blo + size]
            raise AssertionError((lo, size))

        # ---- remaining constants on the gpsimd DMA queue ---------------
        nc.gpsimd.dma_start(
            w_all[:, 2:4, :].rearrange("p w f -> p (w f)").rearrange(
                "p (c f) -> p c f", c=2),
            bass.AP(tensor=w_t, offset=C * C, ap=[[C, P], [P * C, 2], [1, C]]),
        )

        # b_all columns 0,1 = c1 halves
        b_all = const.tile([P, 2], F32, tag="b_all", name="b_all")
        nc.gpsimd.dma_start(
            b_all[:], bass.AP(tensor=b_t, offset=0, ap=[[1, P], [P, 2]]),
        )
        bv_sb = w_all[:, 4:6, :].rearrange("p a b -> p (a b)").bitcast(F32)
        nc.gpsimd.dma_start(
            bv_sb[:], bass.AP(tensor=bv_t, offset=0, ap=[[0, P], [1, C]])
        )

        # ---- persistent activations -----------------------------------
        # u laid out [p, ci, j]: dim1 = the two c-halves (score contraction)
        u_bf = big.tile([P, 2, NJ], BF16, tag="u_bf", name="u_bf")
        v_bf = big.tile([P, NJT, C], BF16, tag="v_bf", name="v_bf")
        # attn [p, jt, i]
        attn = big.tile([P, NJT, N], BF16, tag="attn", name="attn")
        # d_all cols 0:64 = per-(jt,iq) exp sums, 64:80 = D, 80:96 = 1/D
        d_all = big.tile([P, 96], F32, tag="d_all", name="d_all")
        dsum_all = d_all[:, 64:96]

        # ---- warmups: run while the x DMA streams in -------------------
        # dummy matmuls lift the PE HAM clock-gate to 8/8 before real work
        # arrives; a dummy Exp pulls the ~2.7us ACT table load off the
        # critical path of the first score tile.
        with tc.tile_pool(name="warm", bufs=1) as wp_pool, \
             tc.tile_pool(name="warm_ps", bufs=1, space="PSUM") as wpp:
            wsb = wp_pool.tile([P, 512], BF16, tag="wsb", name="wsb")
            wex = wsb[:, 508:509]
            wps = wpp.tile([P, 512], F32, tag="wps", name="wps")
            nc.vector.memset(wsb[:], 0.0)
            # small warms: ramp the PE clock without hogging the queue
            # (the real work is gated on the x/w DMAs anyway)
            for _ in range(6):
                nc.tensor.matmul(wps[:, 0:P], wsb[:, 0:P], wsb[:, 0:P],
                                 start=True, stop=True)
            nc.scalar.activation(wex[:], wps[:, 0:1], AF.Exp, scale=0.0)

        psp = top.enter_context(tc.tile_pool(name="ps_s", bufs=2, space="PSUM"))

        def s_tile(jt, iq):
            # one [128,1024] score tile + exp(+accum) into the attn store
            ps = psp.tile([P, 1024], F32, tag="s", name="s_ps")
            for ci in range(2):
                lhs = u_bf[:, ci, jt * P:(jt + 1) * P]
                for t in range(2):
                    nc.tensor.matmul(
                        ps[:, t * 512:(t + 1) * 512], lhs,
                        xsl(ci, iq * 1024 + t * 512, 512),
                        start=(ci == 0), stop=(ci == 1),
                    )
            nc.scalar.activation(
                attn[:, jt, iq * 1024:(iq + 1) * 1024], ps[:],
                AF.Exp, scale=float(SCALE),
                accum_out=d_all[:, jt * 4 + iq: jt * 4 + iq + 1],
            )

        # ---- phase 1: u projection, early score tiles, v projection ----
        # u first (it gates the scores), then 12 pre-emitted score tiles
        # (they only need x blocks 0-2) so ACT's exp stream starts early,
        # then the v projections (not needed until the first 1/D scaling).
        with tc.tile_pool(name="ps_qkv", bufs=4, space="PSUM") as pq:
            for blk in range(2):
                for ch in range(2):
                    pss = [pq.tile([P, 512], F32, tag="ps", name="ps") for _ in range(2)]
                    for ci in range(2):
                        lhs = w_all[:, ci, ch * P:(ch + 1) * P]
                        for t2 in range(2):
                            t = blk * 2 + t2
                            nc.tensor.matmul(
                                pss[t2][:], lhs,
                                xsl(ci, t * 512, 512),
                                start=(ci == 0), stop=(ci == 1),
                            )
                    for t2 in range(2):
                        t = blk * 2 + t2
                        nc.vector.tensor_scalar_add(
                            u_bf[:, ch, t * 512:(t + 1) * 512], pss[t2][:],
                            b_all[:, ch:ch + 1])

            # pre-emitted score tiles interleaved with the v projections:
            # the in-order PE queue would otherwise stall on the score
            # PSUM rotation (ACT paces the exps) with the v work stuck
            # behind it.
            def v_group(jtg):
                pss = [pq.tile([P, C], F32, tag="ps", name="ps") for _ in range(4)]
                for ci in range(2):
                    for t in range(4):
                        jt = jtg * 4 + t
                        nc.tensor.matmul(
                            pss[t][:],
                            xsl(ci, jt * P, P),
                            w_all[:, 2 + ci, :],
                            start=(ci == 0), stop=(ci == 1),
                        )
                for t in range(4):
                    nc.vector.tensor_add(
                        v_bf[:, jtg * 4 + t, :], pss[t][:], bv_sb[:]
                    )

            pre = [(jt0, iq0) for jt0 in range(6) for iq0 in range(2)]
            for n, (jt0, iq0) in enumerate(pre):
                s_tile(jt0, iq0)
                if n in (1, 3, 5, 7):
                    v_group((n - 1) // 2)

        # ---- phase 2+3 fused: scores/exp interleaved with y accum ------
        with tc.tile_pool(name="yaccp", bufs=1) as yp, \
             tc.tile_pool(name="ps_o", bufs=2, space="PSUM") as po, \
             tc.tile_pool(name="ysb", bufs=2) as ysb_pool:
            y_acc = yp.tile([P, 8, 1024], F32, tag="y_acc", name="y_acc")

            def out_chain(g, idx):
                # one accumulation chain: the group's jts into (iq, ch)
                iq, ch = divmod(idx, 2)
                j0, j1 = GRP[g]
                ops = po.tile([P, 1024], F32, tag="og", name="og")
                for j2 in range(j0, j1):
                    lhs = v_bf[:, j2, ch * P:(ch + 1) * P]
                    for t in range(2):
                        nc.tensor.matmul(
                            ops[:, t * 512:(t + 1) * 512], lhs,
                            attn[:, j2, iq * 1024 + t * 512: iq * 1024 + (t + 1) * 512],
                            start=(j2 == j0), stop=(j2 == j1 - 1),
                        )
                if g == 0:
                    nc.vector.tensor_copy(y_acc[:, idx, :], ops[:])
                elif g < 4:
                    nc.vector.tensor_add(y_acc[:, idx, :], ops[:], y_acc[:, idx, :])
                else:
                    y_sb = ysb_pool.tile([P, 1024], F32, tag="ysb", name="ysb")
                    nc.vector.tensor_add(y_sb[:], ops[:], y_acc[:, idx, :])
                    nc.sync.dma_start(
                        y_t.ap()[ch * P:(ch + 1) * P, iq * 1024:(iq + 1) * 1024],
                        y_sb[:],
                    )

            def d_scale(jt):
                # per-jt denominator (sum the 4 chunk sums) + vp scaling
                nc.vector.reduce_sum(
                    dsum_all[:, jt:jt + 1], d_all[:, jt * 4:jt * 4 + 4],
                    axis=mybir.AxisListType.X,
                )
                nc.vector.reciprocal(
                    dsum_all[:, 16 + jt:17 + jt], dsum_all[:, jt:jt + 1]
                )
                nc.vector.tensor_scalar_mul(
                    v_bf[:, jt, :], v_bf[:, jt, :],
                    dsum_all[:, 16 + jt:17 + jt],
                )

            for jt in range(14):
                for iq in range(4):
                    if jt < 6 and iq < 2:
                        continue  # pre-emitted during the projection phase
                    s_tile(jt, iq)
                # spread jt14/15's first chunks early so their D closes
                # right after their last chunk, shrinking the tail
                if jt == 12:
                    s_tile(14, 0)
                    s_tile(15, 0)
                elif jt == 13:
                    s_tile(14, 1)
                    s_tile(15, 1)
                d_scale(jt)
                if 6 <= jt <= 9:
                    out_chain(0, 2 * (jt - 6))
                    out_chain(0, 2 * (jt - 6) + 1)
                elif 10 <= jt <= 13:
                    out_chain(1, 2 * (jt - 10))
                    out_chain(1, 2 * (jt - 10) + 1)
            # jt14 block
            s_tile(14, 2)
            s_tile(14, 3)
            d_scale(14)
            for idx in range(8):
                out_chain(2, idx)
            # jt15 block: group-3 (jt14) chains run under jt15's exp stream
            s_tile(15, 2)
            for idx in range(4):
                out_chain(3, idx)
            s_tile(15, 3)
            for idx in range(4, 8):
                out_chain(3, idx)
            d_scale(15)
            for idx in range(8):
                out_chain(4, idx)

_nc_cache = None
LAST_EXEC_TIME_NS = None


def _get_nc():
    global _nc_cache
    if _nc_cache is None:
        _nc_cache = _build_module()
    return _nc_cache


def kernel(x, wq, bq, wk, bk, wv, bv, wp, bp):
    global LAST_EXEC_TIME_NS
    nc = _get_nc()

    import ml_dtypes
    bf = ml_dtypes.bfloat16
    x = np.asarray(x, dtype=np.float32).reshape(B, C, N).astype(bf)
    wq32 = np.asarray(wq, dtype=np.float32)
    wk32 = np.asarray(wk, dtype=np.float32)
    wv32 = np.asarray(wv, dtype=np.float32)
    wp32 = np.asarray(wp, dtype=np.float32)
    a_m = wq32.T @ wk32                   # fold q away: s = x^T (A x + c1) + const_j
    c1 = wq32.T @ np.asarray(bk, dtype=np.float32)
    w2 = wp32 @ wv32                      # fold the output projection into v
    wT = np.ascontiguousarray(np.stack([a_m.T, w2.T])).astype(bf)
    b1 = np.ascontiguousarray(c1.reshape(1, C))
    bv2 = np.ascontiguousarray(
        (wp32 @ np.asarray(bv, dtype=np.float32)).reshape(1, C))
    bp1 = np.asarray(bp, dtype=np.float32).reshape(C)

    in_maps = []
    for core in range(8):
        b, h = divmod(core, 2)
        xb = x[b] if h == 0 else np.ascontiguousarray(np.roll(x[b], -NJ, axis=1))
        in_maps.append({"x": xb, "wT": wT, "b": b1, "bv": bv2})

    res = bass_utils.run_bass_kernel_spmd(nc, in_maps, core_ids=list(range(8)))
    if res.exec_time_ns is not None:
        LAST_EXEC_TIME_NS = res.exec_time_ns

    y = np.zeros((B, C, N), np.float32)
    for b in range(B):
        y[b] = res.results[2 * b]["y"] + np.roll(res.results[2 * b + 1]["y"], NJ, axis=1)
    y += bp1.reshape(1, C, 1)
    return y.reshape(B, C, 64, 64)


# revision 21
# speedup vs baseline: 1.0006x; 1.0006x over previous
"""Trainium2 Bass kernel for the AttnBlock-style attention module.

Reference computation (note softmax over axis=1, the *i* axis):
    q = wq @ x + bq ; k = wk @ x + bk ; v = wv @ x + bv      (per-pixel 1x1 conv)
    s[b,i,j] = (q[b,:,i] . k[b,:,j]) * C**-0.5
    attn = softmax_i(s)                                      (normalize over i!)
    out[b,c,i] = sum_j attn[b,i,j] v[b,c,j]
    y = wp @ out + bp

Math folding: softmax over i is invariant to per-j constants, so
    s_ij = q_i.k_j = x_i^T (A x_j + c1) + (const_j)    with A = wq^T wk,
                                                       c1 = wq^T bk.
The q projection disappears entirely: the moving side of the score GEMM is
raw x (already in SBUF from the input DMA), and only u = A x + c1 is
projected for the j-columns. Likewise wp is folded into v (w2 = wp@wv).

Sharding: 8 cores = 4 batches x 2 j-halves. The softmax over i is local to a
j-split (it normalizes each attention *column* j over all i). Each core gets x
with its j-half rotated to columns 0..2047 (a pure permutation of the pixel
axis; the host un-rotates the partial output). Each core:
  - computes u (j-half) and v (j-half) with bf16 GEMMs,
  - s_T[j, i] = u^T x  (j on partitions -> softmax reduction is free-axis),
  - attn = exp(s/16) stored unnormalized in bf16; per-j denominators D[j]
    via the fused activation accum_out; 1/D folded into v rows,
  - out_partial[c, i] = sum_{j in half} v_scaled[c,j] attn_T[j,i],
  - y_partial accumulated in f32 SBUF (bias bp added on host).
Host un-rotates and sums the two j-half partials per batch and adds bp.

Out-accumulation is staged in five j-groups (6,4,4,1,1): the chains for a
group are interleaved into later jts' score/exp stream so the PE fills the
ACT-bound stretches, and the tiny trailing groups shrink the post-last-exp
tail to ~4 matmuls per (iq,ch) slice.
"""

import numpy as np

import concourse.bass as bass
import concourse.mybir as mybir
import concourse.tile as tile
from concourse import bacc
from concourse import bass_utils

P = 128
B = 4
C = 256
N = 4096          # 64*64 pixels
NJ = 2048         # j columns per core
NJT = NJ // P     # 16 j tiles
SCALE = 1.0 / np.sqrt(C).item()   # 1/16

F32 = mybir.dt.float32
BF16 = mybir.dt.bfloat16
AF = mybir.ActivationFunctionType

# j-tile boundaries of the five out-accumulation groups
GRP = [(0, 6), (6, 10), (10, 14), (14, 15), (15, 16)]


def _build_module():
    nc = bacc.Bacc("TRN2", target_bir_lowering=False, debug=False, num_devices=8)

    x_t = nc.dram_tensor("x", [C, N], BF16, kind="ExternalInput")
    w_t = nc.dram_tensor("wT", [2, C, C], BF16, kind="ExternalInput")  # A.T, (wp@wv).T
    b_t = nc.dram_tensor("b", [1, C], F32, kind="ExternalInput")      # c1 = wq.T @ bk
    bv_t = nc.dram_tensor("bv", [1, C], F32, kind="ExternalInput")    # wp@bv
    y_t = nc.dram_tensor("y", [C, N], F32, kind="ExternalOutput")

    with tile.TileContext(nc) as tc:
        _emit(nc, tc, x_t, w_t, b_t, bv_t, y_t)
    nc.compile()
    return nc


def _emit(nc, tc, x_t, w_t, b_t, bv_t, y_t):
    from contextlib import ExitStack

    with ExitStack() as top:
        const = top.enter_context(tc.tile_pool(name="const", bufs=1))
        big = top.enter_context(tc.tile_pool(name="big", bufs=1))

        # ---- u's weight first on the sync queue, then the x blocks ------
        # w_all[:, 2*w + ci, :] = rows ci*128.. of weight w's transpose
        # slots 0,1: A.T chunks; 2,3: w2.T chunks; 4,5: bv as f32 bits
        w_all = const.tile([P, 6, C], BF16, tag="w_all", name="w_all")
        nc.sync.dma_start(
            w_all[:, 0:2, :].rearrange("p w f -> p (w f)").rearrange(
                "p (c f) -> p c f", c=2),
            bass.AP(tensor=w_t, offset=0, ap=[[C, P], [P * C, 2], [1, C]]),
        )

        xp = top.enter_context(tc.tile_pool(name="xload", bufs=1))
        XBLK = [(0, 512), (512, 512), (1024, 1024), (2048, 1024), (3072, 1024)]
        xb = [xp.tile([P, 2, w], BF16, tag=f"xb{b}", name=f"xb{b}")
              for b, (lo, w) in enumerate(XBLK)]
        for b, (lo, w) in enumerate(XBLK):
            if b < 2:
                # early blocks gate the projections: pure-2D patterns keep
                # the descriptor count (and thus transfer latency) down
                for ci in range(2):
                    nc.sync.dma_start(
                        xb[b][:, ci, :],
                        bass.AP(tensor=x_t, offset=ci * P * N + lo,
                                ap=[[N, P], [1, w]]),
                    )
            else:
                nc.sync.dma_start(
                    xb[b][:],
                    bass.AP(tensor=x_t, offset=lo,
                            ap=[[N, P], [P * N, 2], [1, w]]),
                )

        def xsl(ci, lo, size):
            # x[ci*128:(ci+1)*128, lo:lo+size] as an AP (within one block)
            for b, (blo, w) in enumerate(XBLK):
                if blo <= lo and lo + size <= blo + w:
                    return xb[b][:, ci, lo - blo:lo - blo + size]
            raise AssertionError((lo, size))

        # ---- remaining constants on the gpsimd DMA queue ---------------
        nc.gpsimd.dma_start(
            w_all[:, 2:4, :].rearrange("p w f -> p (w f)").rearrange(
                "p (c f) -> p c f", c=2),
            bass.AP(tensor=w_t, offset=C * C, ap=[[C, P], [P * C, 2], [1, C]]),
        )

        # b_all columns 0,1 = c1 halves
        b_all = const.tile([P, 2], F32, tag="b_all", name="b_all")
        nc.gpsimd.dma_start(
            b_all[:], bass.AP(tensor=b_t, offset=0, ap=[[1, P], [P, 2]]),
        )
        bv_sb = w_all[:, 4:6, :].rearrange("p a b -> p (a b)").bitcast(F32)
        nc.gpsimd.dma_start(
            bv_sb[:], bass.AP(tensor=bv_t, offset=0, ap=[[0, P], [1, C]])
        )

        # ---- persistent activations -----------------------------------
        yp = top.enter_context(tc.tile_pool(name="yaccp", bufs=1))
        ysb_pool = top.enter_context(tc.tile_pool(name="ysb", bufs=2))
        # u laid out [p, ci, j]: dim1 = the two c-halves (score contraction)
        u_bf = big.tile([P, 2, NJ], BF16, tag="u_bf", name="u_bf")
        v_bf = big.tile([P, NJT, C], BF16, tag="v_bf", name="v_bf")
        # attn [p, jt, i]
        attn = big.tile([P, NJT, N], BF16, tag="attn", name="attn")
        # d_all cols 0:64 = per-(jt,iq) exp sums, 64:80 = D, 80:96 = 1/D
        d_all = big.tile([P, 96], F32, tag="d_all", name="d_all")
        dsum_all = d_all[:, 64:96]

        # ---- warmups: run while the x DMA streams in -------------------
        # dummy matmuls lift the PE HAM clock-gate to 8/8 before real work
        # arrives; a dummy Exp pulls the ~2.7us ACT table load off the
        # critical path of the first score tile.
        with tc.tile_pool(name="warm", bufs=1) as wp_pool, \
             tc.tile_pool(name="warm_ps", bufs=1, space="PSUM") as wpp:
            wsb = wp_pool.tile([P, 512], BF16, tag="wsb", name="wsb")
            wex = wsb[:, 508:509]
            wps = wpp.tile([P, 512], F32, tag="wps", name="wps")
            nc.vector.memset(wsb[:], 0.0)
            # small warms: ramp the PE clock without hogging the queue
            # (the real work is gated on the x/w DMAs anyway)
            for _ in range(6):
                nc.tensor.matmul(wps[:, 0:P], wsb[:, 0:P], wsb[:, 0:P],
                                 start=True, stop=True)
            nc.scalar.activation(wex[:], wps[:, 0:1], AF.Exp, scale=0.0)

        psp = tc.alloc_tile_pool(name="ps_s", bufs=2, space="PSUM")

        def s_tile(jt, iq):
            # one [128,1024] score tile + exp(+accum) into the attn store
            ps = psp.tile([P, 1024], F32, tag="s", name="s_ps")
            for ci in range(2):
                lhs = u_bf[:, ci, jt * P:(jt + 1) * P]
                for t in range(2):
                    nc.tensor.matmul(
                        ps[:, t * 512:(t + 1) * 512], lhs,
                        xsl(ci, iq * 1024 + t * 512, 512),
                        start=(ci == 0), stop=(ci == 1),
                    )
            nc.scalar.activation(
                attn[:, jt, iq * 1024:(iq + 1) * 1024], ps[:],
                AF.Exp, scale=float(SCALE),
                accum_out=d_all[:, jt * 4 + iq: jt * 4 + iq + 1],
            )

        # ---- phase 1: u projection, early score tiles, v projection ----
        # u first (it gates the scores), then 12 pre-emitted score tiles
        # (they only need x blocks 0-2) so ACT's exp stream starts early,
        # then the v projections (not needed until the first 1/D scaling).
        with tc.tile_pool(name="ps_qkv", bufs=4, space="PSUM") as pq:
            for blk in range(2):
                for ch in range(2):
                    pss = [pq.tile([P, 512], F32, tag="ps", name="ps") for _ in range(2)]
                    for ci in range(2):
                        lhs = w_all[:, ci, ch * P:(ch + 1) * P]
                        for t2 in range(2):
                            t = blk * 2 + t2
                            nc.tensor.matmul(
                                pss[t2][:], lhs,
                                xsl(ci, t * 512, 512),
                                start=(ci == 0), stop=(ci == 1),
                            )
                    for t2 in range(2):
                        t = blk * 2 + t2
                        nc.vector.tensor_scalar_add(
                            u_bf[:, ch, t * 512:(t + 1) * 512], pss[t2][:],
                            b_all[:, ch:ch + 1])

            # pre-emitted score tiles interleaved with the v projections:
            # the in-order PE queue would otherwise stall on the score
            # PSUM rotation (ACT paces the exps) with the v work stuck
            # behind it.
            def v_group(jtg):
                pss = [pq.tile([P, C], F32, tag="ps", name="ps") for _ in range(4)]
                for ci in range(2):
                    for t in range(4):
                        jt = jtg * 4 + t
                        nc.tensor.matmul(
                            pss[t][:],
                            xsl(ci, jt * P, P),
                            w_all[:, 2 + ci, :],
                            start=(ci == 0), stop=(ci == 1),
                        )
                for t in range(4):
                    nc.vector.tensor_add(
                        v_bf[:, jtg * 4 + t, :], pss[t][:], bv_sb[:]
                    )

            pre = [(jt0, iq0) for jt0 in range(6) for iq0 in range(2)]
            for n, (jt0, iq0) in enumerate(pre):
                s_tile(jt0, iq0)
                if n in (1, 3, 5, 7):
                    v_group((n - 1) // 2)

        # ---- phase 2+3 fused: scores/exp interleaved with y accum ------
        po = tc.alloc_tile_pool(name="ps_o", bufs=2, space="PSUM")
        if True:
            y_acc = yp.tile([P, 8, 1024], F32, tag="y_acc", name="y_acc")

            def out_chain(g, idx, pool=None):
                # one accumulation chain: the group's jts into (iq, ch)
                iq, ch = divmod(idx, 2)
                j0, j1 = GRP[g]
                ops = (pool or po).tile([P, 1024], F32, tag="og", name="og")
                for j2 in range(j0, j1):
                    lhs = v_bf[:, j2, ch * P:(ch + 1) * P]
                    for t in range(2):
                        nc.tensor.matmul(
                            ops[:, t * 512:(t + 1) * 512], lhs,
                            attn[:, j2, iq * 1024 + t * 512: iq * 1024 + (t + 1) * 512],
                            start=(j2 == j0), stop=(j2 == j1 - 1),
                        )
                if g == 0:
                    nc.vector.tensor_copy(y_acc[:, idx, :], ops[:])
                elif g < 4:
                    nc.vector.tensor_add(y_acc[:, idx, :], ops[:], y_acc[:, idx, :])
                else:
                    y_sb = ysb_pool.tile([P, 1024], F32, tag="ysb", name="ysb")
                    nc.vector.tensor_add(y_sb[:], ops[:], y_acc[:, idx, :])
                    nc.sync.dma_start(
                        y_t.ap()[ch * P:(ch + 1) * P, iq * 1024:(iq + 1) * 1024],
                        y_sb[:],
                    )

            def d_scale(jt):
                # per-jt denominator (sum the 4 chunk sums) + vp scaling
                nc.vector.reduce_sum(
                    dsum_all[:, jt:jt + 1], d_all[:, jt * 4:jt * 4 + 4],
                    axis=mybir.AxisListType.X,
                )
                nc.vector.reciprocal(
                    dsum_all[:, 16 + jt:17 + jt], dsum_all[:, jt:jt + 1]
                )
                nc.vector.tensor_scalar_mul(
                    v_bf[:, jt, :], v_bf[:, jt, :],
                    dsum_all[:, 16 + jt:17 + jt],
                )

            for jt in range(14):
                for iq in range(4):
                    if jt < 6 and iq < 2:
                        continue  # pre-emitted during the projection phase
                    s_tile(jt, iq)
                # spread jt14/15's first chunks early so their D closes
                # right after their last chunk, shrinking the tail
                if jt == 12:
                    s_tile(14, 0)
                    s_tile(15, 0)
                elif jt == 13:
                    s_tile(14, 1)
                    s_tile(15, 1)
                d_scale(jt)
                if 6 <= jt <= 9:
                    out_chain(0, 2 * (jt - 6))
                    out_chain(0, 2 * (jt - 6) + 1)
                elif 10 <= jt <= 13:
                    out_chain(1, 2 * (jt - 10))
                    out_chain(1, 2 * (jt - 10) + 1)
            # jt14 block
            s_tile(14, 2)
            s_tile(14, 3)
            d_scale(14)
            for idx in range(8):
                out_chain(2, idx)
            # jt15 block: group-3 (jt14) chains run under jt15's exp stream
            s_tile(15, 2)
            for idx in range(4):
                out_chain(3, idx)
            s_tile(15, 3)
            for idx in range(4, 8):
                out_chain(3, idx)
            d_scale(15)
            # final group: the score/chain PSUM pools are done — release
            # them and run the last 8 chains on a 4-deep rotation so the
            # matmuls stay dense (the sparse tail otherwise downclocks the
            # PE) and the DVE drains pipeline behind them.
            po.release()
            psp.release()
            po4 = tc.alloc_tile_pool(name="ps_o4", bufs=4, space="PSUM")
            # keep-warm: hold the PE clock up while exp15/scale finish
            warm2 = po4.tile([P, 1024], F32, tag="og", name="og_warm")
            for _ in range(4):
                nc.tensor.matmul(warm2[:, 0:P], u_bf[:, 0, 0:P],
                                 u_bf[:, 0, 0:P], start=True, stop=True)
            for idx in range(8):
                out_chain(4, idx, pool=po4)
            po4.release()

_nc_cache = None
LAST_EXEC_TIME_NS = None


def _get_nc():
    global _nc_cache
    if _nc_cache is None:
        _nc_cache = _build_module()
    return _nc_cache


def kernel(x, wq, bq, wk, bk, wv, bv, wp, bp):
    global LAST_EXEC_TIME_NS
    nc = _get_nc()

    import ml_dtypes
    bf = ml_dtypes.bfloat16
    x = np.asarray(x, dtype=np.float32).reshape(B, C, N).astype(bf)
    wq32 = np.asarray(wq, dtype=np.float32)
    wk32 = np.asarray(wk, dtype=np.float32)
    wv32 = np.asarray(wv, dtype=np.float32)
    wp32 = np.asarray(wp, dtype=np.float32)
    a_m = wq32.T @ wk32                   # fold q away: s = x^T (A x + c1) + const_j
    c1 = wq32.T @ np.asarray(bk, dtype=np.float32)
    w2 = wp32 @ wv32                      # fold the output projection into v
    wT = np.ascontiguousarray(np.stack([a_m.T, w2.T])).astype(bf)
    b1 = np.ascontiguousarray(c1.reshape(1, C))
    bv2 = np.ascontiguousarray(
        (wp32 @ np.asarray(bv, dtype=np.float32)).reshape(1, C))
    bp1 = np.asarray(bp, dtype=np.float32).reshape(C)

    in_maps = []
    for core in range(8):
        b, h = divmod(core, 2)
        xb = x[b] if h == 0 else np.ascontiguousarray(np.roll(x[b], -NJ, axis=1))
        in_maps.append({"x": xb, "wT": wT, "b": b1, "bv": bv2})

    res = bass_utils.run_bass_kernel_spmd(nc, in_maps, core_ids=list(range(8)))
    if res.exec_time_ns is not None:
        LAST_EXEC_TIME_NS = res.exec_time_ns

    y = np.zeros((B, C, N), np.float32)
    for b in range(B):
        y[b] = res.results[2 * b]["y"] + np.roll(res.results[2 * b + 1]["y"], NJ, axis=1)
    y += bp1.reshape(1, C, 1)
    return y.reshape(B, C, 64, 64)


# revision 30
# speedup vs baseline: 1.0073x; 1.0066x over previous
"""Trainium2 Bass kernel for the AttnBlock-style attention module.

Reference computation (note softmax over axis=1, the *i* axis):
    q = wq @ x + bq ; k = wk @ x + bk ; v = wv @ x + bv      (per-pixel 1x1 conv)
    s[b,i,j] = (q[b,:,i] . k[b,:,j]) * C**-0.5
    attn = softmax_i(s)                                      (normalize over i!)
    out[b,c,i] = sum_j attn[b,i,j] v[b,c,j]
    y = wp @ out + bp

Math folding: softmax over i is invariant to per-j constants, so
    s_ij = q_i.k_j = x_i^T (A x_j + c1) + (const_j)    with A = wq^T wk,
                                                       c1 = wq^T bk.
The q projection disappears entirely: the moving side of the score GEMM is
raw x (already in SBUF from the input DMA), and only u = A x + c1 is
projected for the j-columns. Likewise wp is folded into v (w2 = wp@wv).

Sharding: 8 cores = 4 batches x 2 j-halves. The softmax over i is local to a
j-split (it normalizes each attention *column* j over all i). Each core gets x
with its j-half rotated to columns 0..2047 (a pure permutation of the pixel
axis; the host un-rotates the partial output). Each core:
  - computes u (j-half) and v (j-half) with bf16 GEMMs,
  - s_T[j, i] = u^T x  (j on partitions -> softmax reduction is free-axis),
  - attn = exp(s/16) stored unnormalized in bf16; per-j denominators D[j]
    via the fused activation accum_out; 1/D folded into v rows,
  - out_partial[c, i] = sum_{j in half} v_scaled[c,j] attn_T[j,i],
  - y_partial accumulated in f32 SBUF (bias bp added on host).
Host un-rotates and sums the two j-half partials per batch and adds bp.

Out-accumulation is staged in five j-groups (6,4,4,1,1): the chains for a
group are interleaved into later jts' score/exp stream so the PE fills the
ACT-bound stretches, and the tiny trailing groups shrink the post-last-exp
tail to ~4 matmuls per (iq,ch) slice.
"""

import numpy as np

import concourse.bass as bass
import concourse.mybir as mybir
import concourse.tile as tile
from concourse import bacc
from concourse import bass_utils

P = 128
B = 4
C = 256
N = 4096          # 64*64 pixels
NJ = 2048         # j columns per core
NJT = NJ // P     # 16 j tiles
SCALE = 1.0 / np.sqrt(C).item()   # 1/16

F32 = mybir.dt.float32
F32R = mybir.dt.float32r
BF16 = mybir.dt.bfloat16
AF = mybir.ActivationFunctionType

# j-tile boundaries of the five out-accumulation groups
GRP = [(0, 6), (6, 10), (10, 14), (14, 15), (15, 16)]


def _build_module():
    nc = bacc.Bacc("TRN2", target_bir_lowering=False, debug=False, num_devices=8)

    x_t = nc.dram_tensor("x", [C, N], BF16, kind="ExternalInput")
    w_t = nc.dram_tensor("wT", [2, C, C], BF16, kind="ExternalInput")  # A.T, (wp@wv).T
    b_t = nc.dram_tensor("b", [1, C], F32, kind="ExternalInput")      # c1 = wq.T @ bk
    bv_t = nc.dram_tensor("bv", [1, C], F32, kind="ExternalInput")    # wp@bv
    id_t = nc.dram_tensor("ident", [P, P], F32R, kind="ExternalInput")
    y_t = nc.dram_tensor("y", [C, N], F32, kind="ExternalOutput")

    with tile.TileContext(nc) as tc:
        _emit(nc, tc, x_t, w_t, b_t, bv_t, id_t, y_t)
    nc.compile()
    return nc


def _emit(nc, tc, x_t, w_t, b_t, bv_t, id_t, y_t):
    from contextlib import ExitStack

    with ExitStack() as top:
        const = top.enter_context(tc.tile_pool(name="const", bufs=1))
        big = top.enter_context(tc.tile_pool(name="big", bufs=1))

        # ---- u's weight first on the sync queue, then the x blocks ------
        # w_all[:, 2*w + ci, :] = rows ci*128.. of weight w's transpose
        # slots 0,1: A.T chunks; 2,3: w2.T chunks; 4,5: bv as f32 bits
        w_all = const.tile([P, 6, C], BF16, tag="w_all", name="w_all")
        nc.sync.dma_start(
            w_all[:, 0:2, :].rearrange("p w f -> p (w f)").rearrange(
                "p (c f) -> p c f", c=2),
            bass.AP(tensor=w_t, offset=0, ap=[[C, P], [P * C, 2], [1, C]]),
        )

        xp = top.enter_context(tc.tile_pool(name="xload", bufs=1))
        XBLK = [(0, 512), (512, 512), (1024, 1024), (2048, 1024), (3072, 1024)]
        xb = [xp.tile([P, 2, w], BF16, tag=f"xb{b}", name=f"xb{b}")
              for b, (lo, w) in enumerate(XBLK)]
        for b, (lo, w) in enumerate(XBLK):
            if b < 2:
                # early blocks gate the projections: pure-2D patterns keep
                # the descriptor count (and thus transfer latency) down
                for ci in range(2):
                    nc.sync.dma_start(
                        xb[b][:, ci, :],
                        bass.AP(tensor=x_t, offset=ci * P * N + lo,
                                ap=[[N, P], [1, w]]),
                    )
            else:
                nc.sync.dma_start(
                    xb[b][:],
                    bass.AP(tensor=x_t, offset=lo,
                            ap=[[N, P], [P * N, 2], [1, w]]),
                )

        def xsl(ci, lo, size):
            # x[ci*128:(ci+1)*128, lo:lo+size] as an AP (within one block)
            for b, (blo, w) in enumerate(XBLK):
                if blo <= lo and lo + size <= blo + w:
                    return xb[b][:, ci, lo - blo:lo - blo + size]
            raise AssertionError((lo, size))

        # ---- remaining constants on the gpsimd DMA queue ---------------
        nc.gpsimd.dma_start(
            w_all[:, 2:4, :].rearrange("p w f -> p (w f)").rearrange(
                "p (c f) -> p c f", c=2),
            bass.AP(tensor=w_t, offset=C * C, ap=[[C, P], [P * C, 2], [1, C]]),
        )

        # b_all columns 0,1 = c1 halves
        b_all = const.tile([P, 2], F32, tag="b_all", name="b_all")
        nc.gpsimd.dma_start(
            b_all[:], bass.AP(tensor=b_t, offset=0, ap=[[1, P], [P, 2]]),
        )
        bv_sb = w_all[:, 4:6, :].rearrange("p a b -> p (a b)").bitcast(F32)
        nc.gpsimd.dma_start(
            bv_sb[:], bass.AP(tensor=bv_t, offset=0, ap=[[0, P], [1, C]])
        )
        # f32 identity for the tail's y_acc-into-PSUM merge matmuls
        idn = const.tile([P, P], F32R, tag="idn", name="idn")
        nc.gpsimd.dma_start(
            idn[:], bass.AP(tensor=id_t, offset=0, ap=[[P, P], [1, P]])
        )

        # ---- persistent activations -----------------------------------
        yp = top.enter_context(tc.tile_pool(name="yaccp", bufs=1))
        ysb_pool = top.enter_context(tc.tile_pool(name="ysb", bufs=2))
        # u laid out [p, ci, j]: dim1 = the two c-halves (score contraction)
        u_bf = big.tile([P, 2, NJ], BF16, tag="u_bf", name="u_bf")
        v_bf = big.tile([P, NJT, C], BF16, tag="v_bf", name="v_bf")
        # attn [p, jt, i]
        attn = big.tile([P, NJT, N], BF16, tag="attn", name="attn")
        # d_all cols 0:64 = per-(jt,iq) exp sums, 64:80 = D, 80:96 = 1/D
        d_all = big.tile([P, 96], F32, tag="d_all", name="d_all")
        dsum_all = d_all[:, 64:96]

        # ---- warmups: run while the x DMA streams in -------------------
        # dummy matmuls lift the PE HAM clock-gate to 8/8 before real work
        # arrives; a dummy Exp pulls the ~2.7us ACT table load off the
        # critical path of the first score tile.
        with tc.tile_pool(name="warm", bufs=1) as wp_pool, \
             tc.tile_pool(name="warm_ps", bufs=1, space="PSUM") as wpp:
            wsb = wp_pool.tile([P, 512], BF16, tag="wsb", name="wsb")
            wex = wsb[:, 508:509]
            wps = wpp.tile([P, 512], F32, tag="wps", name="wps")
            nc.vector.memset(wsb[:], 0.0)
            # small warms: ramp the PE clock without hogging the queue
            # (the real work is gated on the x/w DMAs anyway)
            for _ in range(6):
                nc.tensor.matmul(wps[:, 0:P], wsb[:, 0:P], wsb[:, 0:P],
                                 start=True, stop=True)
            nc.scalar.activation(wex[:], wps[:, 0:1], AF.Exp, scale=0.0)

        psp = tc.alloc_tile_pool(name="ps_s", bufs=2, space="PSUM")

        def s_tile(jt, iq):
            # one [128,1024] score tile + exp(+accum) into the attn store
            ps = psp.tile([P, 1024], F32, tag="s", name="s_ps")
            for ci in range(2):
                lhs = u_bf[:, ci, jt * P:(jt + 1) * P]
                for t in range(2):
                    nc.tensor.matmul(
                        ps[:, t * 512:(t + 1) * 512], lhs,
                        xsl(ci, iq * 1024 + t * 512, 512),
                        start=(ci == 0), stop=(ci == 1),
                    )
            nc.scalar.activation(
                attn[:, jt, iq * 1024:(iq + 1) * 1024], ps[:],
                AF.Exp, scale=float(SCALE),
                accum_out=d_all[:, jt * 4 + iq: jt * 4 + iq + 1],
            )

        # ---- phase 1: u projection, early score tiles, v projection ----
        # u first (it gates the scores), then 12 pre-emitted score tiles
        # (they only need x blocks 0-2) so ACT's exp stream starts early,
        # then the v projections (not needed until the first 1/D scaling).
        with tc.tile_pool(name="ps_qkv", bufs=4, space="PSUM") as pq:
            for blk in range(2):
                for ch in range(2):
                    pss = [pq.tile([P, 512], F32, tag="ps", name="ps") for _ in range(2)]
                    for ci in range(2):
                        lhs = w_all[:, ci, ch * P:(ch + 1) * P]
                        for t2 in range(2):
                            t = blk * 2 + t2
                            nc.tensor.matmul(
                                pss[t2][:], lhs,
                                xsl(ci, t * 512, 512),
                                start=(ci == 0), stop=(ci == 1),
                            )
                    for t2 in range(2):
                        t = blk * 2 + t2
                        nc.vector.tensor_scalar_add(
                            u_bf[:, ch, t * 512:(t + 1) * 512], pss[t2][:],
                            b_all[:, ch:ch + 1])

            # pre-emitted score tiles interleaved with the v projections:
            # the in-order PE queue would otherwise stall on the score
            # PSUM rotation (ACT paces the exps) with the v work stuck
            # behind it.
            def v_group(jtg):
                pss = [pq.tile([P, C], F32, tag="ps", name="ps") for _ in range(4)]
                for ci in range(2):
                    for t in range(4):
                        jt = jtg * 4 + t
                        nc.tensor.matmul(
                            pss[t][:],
                            xsl(ci, jt * P, P),
                            w_all[:, 2 + ci, :],
                            start=(ci == 0), stop=(ci == 1),
                        )
                for t in range(4):
                    nc.vector.tensor_add(
                        v_bf[:, jtg * 4 + t, :], pss[t][:], bv_sb[:]
                    )

            pre = [(jt0, iq0) for jt0 in range(6) for iq0 in range(2)]
            for n, (jt0, iq0) in enumerate(pre):
                s_tile(jt0, iq0)
                if n in (1, 3, 5, 7):
                    v_group((n - 1) // 2)

        # ---- phase 2+3 fused: scores/exp interleaved with y accum ------
        po = tc.alloc_tile_pool(name="ps_o", bufs=2, space="PSUM")
        if True:
            # f32r so the tail's identity matmuls can consume it directly
            y_acc = yp.tile([P, 8, 1024], F32R, tag="y_acc", name="y_acc")

            def out_chain(g, idx, pool=None):
                # one accumulation chain: the group's jts into (iq, ch)
                iq, ch = divmod(idx, 2)
                j0, j1 = GRP[g]
                ops = (pool or po).tile([P, 1024], F32, tag="og", name="og")
                for j2 in range(j0, j1):
                    lhs = v_bf[:, j2, ch * P:(ch + 1) * P]
                    for t in range(2):
                        nc.tensor.matmul(
                            ops[:, t * 512:(t + 1) * 512], lhs,
                            attn[:, j2, iq * 1024 + t * 512: iq * 1024 + (t + 1) * 512],
                            start=(j2 == j0), stop=(j2 == j1 - 1),
                        )
                if g == 0:
                    nc.vector.tensor_copy(y_acc[:, idx, :], ops[:])
                elif g < 4:
                    nc.vector.tensor_add(y_acc[:, idx, :], ops[:], y_acc[:, idx, :])
                else:
                    y_sb = ysb_pool.tile([P, 1024], F32, tag="ysb", name="ysb")
                    nc.vector.tensor_add(y_sb[:], ops[:], y_acc[:, idx, :])
                    nc.sync.dma_start(
                        y_t.ap()[ch * P:(ch + 1) * P, iq * 1024:(iq + 1) * 1024],
                        y_sb[:],
                    )

            def d_scale(jt):
                # per-jt denominator (sum the 4 chunk sums) + vp scaling
                nc.vector.reduce_sum(
                    dsum_all[:, jt:jt + 1], d_all[:, jt * 4:jt * 4 + 4],
                    axis=mybir.AxisListType.X,
                )
                nc.vector.reciprocal(
                    dsum_all[:, 16 + jt:17 + jt], dsum_all[:, jt:jt + 1]
                )
                nc.vector.tensor_scalar_mul(
                    v_bf[:, jt, :], v_bf[:, jt, :],
                    dsum_all[:, 16 + jt:17 + jt],
                )

            for jt in range(14):
                for iq in range(4):
                    if jt < 6 and iq < 2:
                        continue  # pre-emitted during the projection phase
                    s_tile(jt, iq)
                # spread jt14/15's first chunks early so their D closes
                # right after their last chunk, shrinking the tail
                if jt == 12:
                    s_tile(14, 0)
                    s_tile(15, 0)
                elif jt == 13:
                    s_tile(14, 1)
                    s_tile(15, 1)
                d_scale(jt)
                if 6 <= jt <= 9:
                    out_chain(0, 2 * (jt - 6))
                    out_chain(0, 2 * (jt - 6) + 1)
                elif 10 <= jt <= 13:
                    out_chain(1, 2 * (jt - 10))
                    out_chain(1, 2 * (jt - 10) + 1)
                if jt == 13:
                    # group 2 (jts 10-13) is complete: start half its
                    # chains here so the endgame isn't chain-heavy
                    for idx in range(4):
                        out_chain(2, idx)
            # jt14 block
            s_tile(14, 2)
            s_tile(14, 3)
            d_scale(14)
            for idx in range(4, 8):
                out_chain(2, idx)
            # jt15 block: group-3 (jt14) chains run under jt15's exp stream
            s_tile(15, 2)
            for idx in range(4):
                out_chain(3, idx)
            s_tile(15, 3)
            for idx in range(4, 8):
                out_chain(3, idx)
            d_scale(15)
            # final group (jt15): release the score/chain pools and run the
            # last 8 chains on a 4-deep PSUM rotation. For half the slices
            # the y_acc partial is merged into the chain PSUM with an f32r
            # identity matmul so the drain is a pure copy on the otherwise
            # idle ACT engine; the rest use DVE adds — the two engines
            # drain concurrently.
            po.release()
            psp.release()
            po4 = tc.alloc_tile_pool(name="ps_o4", bufs=4, space="PSUM")
            idn_r = idn[:]
            for idx in range(8):
                iq, ch = divmod(idx, 2)
                on_act = (idx % 2 == 0)
                ops = po4.tile([P, 1024], F32, tag="og", name="og")
                for t in range(2):
                    sl = slice(t * 512, (t + 1) * 512)
                    isl = slice(iq * 1024 + t * 512, iq * 1024 + (t + 1) * 512)
                    if on_act:
                        nc.tensor.matmul(
                            ops[:, sl], idn_r,
                            y_acc[:, idx, sl],
                            start=True, stop=False,
                        )
                        nc.tensor.matmul(
                            ops[:, sl], v_bf[:, 15, ch * P:(ch + 1) * P],
                            attn[:, 15, isl], start=False, stop=True,
                        )
                    else:
                        nc.tensor.matmul(
                            ops[:, sl], v_bf[:, 15, ch * P:(ch + 1) * P],
                            attn[:, 15, isl], start=True, stop=True,
                        )
                y_sb = ysb_pool.tile([P, 1024], F32, tag="ysb", name="ysb")
                if on_act:
                    nc.scalar.copy(y_sb[:], ops[:])
                else:
                    nc.vector.tensor_add(y_sb[:], ops[:], y_acc[:, idx, :])
                nc.sync.dma_start(
                    y_t.ap()[ch * P:(ch + 1) * P, iq * 1024:(iq + 1) * 1024],
                    y_sb[:],
                )
            po4.release()

_nc_cache = None
LAST_EXEC_TIME_NS = None


def _get_nc():
    global _nc_cache
    if _nc_cache is None:
        _nc_cache = _build_module()
    return _nc_cache


def kernel(x, wq, bq, wk, bk, wv, bv, wp, bp):
    global LAST_EXEC_TIME_NS
    nc = _get_nc()

    import ml_dtypes
    bf = ml_dtypes.bfloat16
    x = np.asarray(x, dtype=np.float32).reshape(B, C, N).astype(bf)
    wq32 = np.asarray(wq, dtype=np.float32)
    wk32 = np.asarray(wk, dtype=np.float32)
    wv32 = np.asarray(wv, dtype=np.float32)
    wp32 = np.asarray(wp, dtype=np.float32)
    a_m = wq32.T @ wk32                   # fold q away: s = x^T (A x + c1) + const_j
    c1 = wq32.T @ np.asarray(bk, dtype=np.float32)
    w2 = wp32 @ wv32                      # fold the output projection into v
    wT = np.ascontiguousarray(np.stack([a_m.T, w2.T])).astype(bf)
    b1 = np.ascontiguousarray(c1.reshape(1, C))
    bv2 = np.ascontiguousarray(
        (wp32 @ np.asarray(bv, dtype=np.float32)).reshape(1, C))
    bp1 = np.asarray(bp, dtype=np.float32).reshape(C)

    ident = np.eye(P, dtype=np.float32)
    in_maps = []
    for core in range(8):
        b, h = divmod(core, 2)
        xb = x[b] if h == 0 else np.ascontiguousarray(np.roll(x[b], -NJ, axis=1))
        in_maps.append({"x": xb, "wT": wT, "b": b1, "bv": bv2, "ident": ident})

    res = bass_utils.run_bass_kernel_spmd(nc, in_maps, core_ids=list(range(8)))
    if res.exec_time_ns is not None:
        LAST_EXEC_TIME_NS = res.exec_time_ns

    y = np.zeros((B, C, N), np.float32)
    for b in range(B):
        y[b] = res.results[2 * b]["y"] + np.roll(res.results[2 * b + 1]["y"], NJ, axis=1)
    y += bp1.reshape(1, C, 1)
    return y.reshape(B, C, 64, 64)


# revision 35
# speedup vs baseline: 1.0495x; 1.0419x over previous
"""Trainium2 Bass kernel for the AttnBlock-style attention module.

Reference computation (note softmax over axis=1, the *i* axis):
    q = wq @ x + bq ; k = wk @ x + bk ; v = wv @ x + bv      (per-pixel 1x1 conv)
    s[b,i,j] = (q[b,:,i] . k[b,:,j]) * C**-0.5
    attn = softmax_i(s)                                      (normalize over i!)
    out[b,c,i] = sum_j attn[b,i,j] v[b,c,j]
    y = wp @ out + bp

Math folding: softmax over i is invariant to per-j constants, so
    s_ij = q_i.k_j = x_i^T (A x_j + c1) + (const_j)    with A = wq^T wk,
                                                       c1 = wq^T bk.
The q projection disappears entirely: the moving side of the score GEMM is
raw x (already in SBUF from the input DMA), and only u = A x + c1 is
projected for the j-columns. Likewise wp is folded into v (w2 = wp@wv).

Sharding: 8 cores = 4 batches x 2 j-halves. The softmax over i is local to a
j-split (it normalizes each attention *column* j over all i). Each core gets x
with its j-half rotated to columns 0..2047 (a pure permutation of the pixel
axis; the host un-rotates the partial output). Each core:
  - computes u (j-half) and v (j-half) with bf16 GEMMs,
  - s_T[j, i] = u^T x  (j on partitions -> softmax reduction is free-axis),
  - attn = exp(s/16) stored unnormalized in bf16; per-j denominators D[j]
    via the fused activation accum_out; 1/D folded into v rows,
  - out_partial[c, i] = sum_{j in half} v_scaled[c,j] attn_T[j,i],
  - y_partial accumulated in f32 SBUF (bias bp added on host).
Host un-rotates and sums the two j-half partials per batch and adds bp.

Out-accumulation is staged in five j-groups (6,4,4,1,1): the chains for a
group are interleaved into later jts' score/exp stream so the PE fills the
ACT-bound stretches, and the tiny trailing groups shrink the post-last-exp
tail to ~4 matmuls per (iq,ch) slice.
"""

import numpy as np

import concourse.bass as bass
import concourse.mybir as mybir
import concourse.tile as tile
from concourse import bacc
from concourse import bass_utils

P = 128
B = 4
C = 256
N = 4096          # 64*64 pixels
NJ = 2048         # j columns per core
NJT = NJ // P     # 16 j tiles
SCALE = 1.0 / np.sqrt(C).item()   # 1/16

F32 = mybir.dt.float32
F32R = mybir.dt.float32r
BF16 = mybir.dt.bfloat16
AF = mybir.ActivationFunctionType

# j-tile boundaries of the out-accumulation groups (last one is inlined)
GRP = [(0, 6), (6, 10), (10, 14), (14, 16)]


def _build_module():
    nc = bacc.Bacc("TRN2", target_bir_lowering=False, debug=False, num_devices=8)

    x_t = nc.dram_tensor("x", [C, N], BF16, kind="ExternalInput")
    w_t = nc.dram_tensor("wT", [2, C, C], BF16, kind="ExternalInput")  # A.T, (wp@wv).T
    b_t = nc.dram_tensor("b", [1, C], F32, kind="ExternalInput")      # c1 = wq.T @ bk
    bv_t = nc.dram_tensor("bv", [1, C], F32, kind="ExternalInput")    # wp@bv
    id_t = nc.dram_tensor("ident", [P, P], F32R, kind="ExternalInput")
    y_t = nc.dram_tensor("y", [C, N], F32, kind="ExternalOutput")

    with tile.TileContext(nc) as tc:
        _emit(nc, tc, x_t, w_t, b_t, bv_t, id_t, y_t)
    nc.compile()
    return nc


def _emit(nc, tc, x_t, w_t, b_t, bv_t, id_t, y_t):
    from contextlib import ExitStack

    with ExitStack() as top:
        const = top.enter_context(tc.tile_pool(name="const", bufs=1))
        big = top.enter_context(tc.tile_pool(name="big", bufs=1))

        # ---- u's weight first on the sync queue, then the x blocks ------
        # w_all[:, 2*w + ci, :] = rows ci*128.. of weight w's transpose
        # slots 0,1: A.T chunks; 2,3: w2.T chunks; 4,5: bv as f32 bits
        w_all = const.tile([P, 6, C], BF16, tag="w_all", name="w_all")
        nc.sync.dma_start(
            w_all[:, 0:2, :].rearrange("p w f -> p (w f)").rearrange(
                "p (c f) -> p c f", c=2),
            bass.AP(tensor=w_t, offset=0, ap=[[C, P], [P * C, 2], [1, C]]),
        )

        xp = top.enter_context(tc.tile_pool(name="xload", bufs=1))
        XBLK = [(0, 512), (512, 512), (1024, 1024), (2048, 1024), (3072, 1024)]
        xb = [xp.tile([P, 2, w], BF16, tag=f"xb{b}", name=f"xb{b}")
              for b, (lo, w) in enumerate(XBLK)]
        for b, (lo, w) in enumerate(XBLK):
            if b < 2:
                # early blocks gate the projections: pure-2D patterns keep
                # the descriptor count (and thus transfer latency) down
                for ci in range(2):
                    nc.sync.dma_start(
                        xb[b][:, ci, :],
                        bass.AP(tensor=x_t, offset=ci * P * N + lo,
                                ap=[[N, P], [1, w]]),
                    )
            else:
                nc.sync.dma_start(
                    xb[b][:],
                    bass.AP(tensor=x_t, offset=lo,
                            ap=[[N, P], [P * N, 2], [1, w]]),
                )

        def xsl(ci, lo, size):
            # x[ci*128:(ci+1)*128, lo:lo+size] as an AP (within one block)
            for b, (blo, w) in enumerate(XBLK):
                if blo <= lo and lo + size <= blo + w:
                    return xb[b][:, ci, lo - blo:lo - blo + size]
            raise AssertionError((lo, size))

        # ---- remaining constants on the gpsimd DMA queue ---------------
        nc.gpsimd.dma_start(
            w_all[:, 2:4, :].rearrange("p w f -> p (w f)").rearrange(
                "p (c f) -> p c f", c=2),
            bass.AP(tensor=w_t, offset=C * C, ap=[[C, P], [P * C, 2], [1, C]]),
        )

        # b_all columns 0,1 = c1 halves
        b_all = const.tile([P, 2], F32, tag="b_all", name="b_all")
        nc.gpsimd.dma_start(
            b_all[:], bass.AP(tensor=b_t, offset=0, ap=[[1, P], [P, 2]]),
        )
        bv_sb = w_all[:, 4:6, :].rearrange("p a b -> p (a b)").bitcast(F32)
        nc.gpsimd.dma_start(
            bv_sb[:], bass.AP(tensor=bv_t, offset=0, ap=[[0, P], [1, C]])
        )
        # f32 identity for the tail's y_acc-into-PSUM merge matmuls
        idn = const.tile([P, P], F32R, tag="idn", name="idn")
        nc.gpsimd.dma_start(
            idn[:], bass.AP(tensor=id_t, offset=0, ap=[[P, P], [1, P]])
        )

        # ---- persistent activations -----------------------------------
        yp = top.enter_context(tc.tile_pool(name="yaccp", bufs=1))
        ysb_pool = top.enter_context(tc.tile_pool(name="ysb", bufs=2))
        # u laid out [p, ci, j]: dim1 = the two c-halves (score contraction)
        u_bf = big.tile([P, 2, NJ], BF16, tag="u_bf", name="u_bf")
        v_bf = big.tile([P, NJT, C], BF16, tag="v_bf", name="v_bf")
        # attn [p, jt, i]
        attn = big.tile([P, NJT, N], BF16, tag="attn", name="attn")
        # d_all cols 0:64 = per-(jt,iq) exp sums, 64:80 = D, 80:96 = 1/D
        d_all = big.tile([P, 96], F32, tag="d_all", name="d_all")
        dsum_all = d_all[:, 64:96]

        # ---- warmups: run while the x DMA streams in -------------------
        # dummy matmuls lift the PE HAM clock-gate to 8/8 before real work
        # arrives; a dummy Exp pulls the ~2.7us ACT table load off the
        # critical path of the first score tile.
        with tc.tile_pool(name="warm", bufs=1) as wp_pool, \
             tc.tile_pool(name="warm_ps", bufs=1, space="PSUM") as wpp:
            wsb = wp_pool.tile([P, 512], BF16, tag="wsb", name="wsb")
            wex = wsb[:, 508:509]
            wps = wpp.tile([P, 512], F32, tag="wps", name="wps")
            nc.vector.memset(wsb[:], 0.0)
            # small warms: ramp the PE clock without hogging the queue
            # (the real work is gated on the x/w DMAs anyway)
            for _ in range(6):
                nc.tensor.matmul(wps[:, 0:P], wsb[:, 0:P], wsb[:, 0:P],
                                 start=True, stop=True)
            nc.scalar.activation(wex[:], wps[:, 0:1], AF.Exp, scale=0.0)

        psp = tc.alloc_tile_pool(name="ps_s", bufs=2, space="PSUM")

        def s_tile(jt, iq):
            # one [128,1024] score tile + exp(+accum) into the attn store
            ps = psp.tile([P, 1024], F32, tag="s", name="s_ps")
            for ci in range(2):
                lhs = u_bf[:, ci, jt * P:(jt + 1) * P]
                for t in range(2):
                    nc.tensor.matmul(
                        ps[:, t * 512:(t + 1) * 512], lhs,
                        xsl(ci, iq * 1024 + t * 512, 512),
                        start=(ci == 0), stop=(ci == 1),
                    )
            nc.scalar.activation(
                attn[:, jt, iq * 1024:(iq + 1) * 1024], ps[:],
                AF.Exp, scale=float(SCALE),
                accum_out=d_all[:, jt * 4 + iq: jt * 4 + iq + 1],
            )

        # ---- phase 1: u projection, early score tiles, v projection ----
        # u first (it gates the scores), then 12 pre-emitted score tiles
        # (they only need x blocks 0-2) so ACT's exp stream starts early,
        # then the v projections (not needed until the first 1/D scaling).
        with tc.tile_pool(name="ps_qkv", bufs=4, space="PSUM") as pq:
            for blk in range(2):
                for ch in range(2):
                    pss = [pq.tile([P, 512], F32, tag="ps", name="ps") for _ in range(2)]
                    for ci in range(2):
                        lhs = w_all[:, ci, ch * P:(ch + 1) * P]
                        for t2 in range(2):
                            t = blk * 2 + t2
                            nc.tensor.matmul(
                                pss[t2][:], lhs,
                                xsl(ci, t * 512, 512),
                                start=(ci == 0), stop=(ci == 1),
                            )
                    for t2 in range(2):
                        t = blk * 2 + t2
                        nc.vector.tensor_scalar_add(
                            u_bf[:, ch, t * 512:(t + 1) * 512], pss[t2][:],
                            b_all[:, ch:ch + 1])

            # pre-emitted score tiles interleaved with the v projections:
            # the in-order PE queue would otherwise stall on the score
            # PSUM rotation (ACT paces the exps) with the v work stuck
            # behind it.
            def v_group(jtg):
                pss = [pq.tile([P, C], F32, tag="ps", name="ps") for _ in range(4)]
                for ci in range(2):
                    for t in range(4):
                        jt = jtg * 4 + t
                        nc.tensor.matmul(
                            pss[t][:],
                            xsl(ci, jt * P, P),
                            w_all[:, 2 + ci, :],
                            start=(ci == 0), stop=(ci == 1),
                        )
                for t in range(4):
                    nc.vector.tensor_add(
                        v_bf[:, jtg * 4 + t, :], pss[t][:], bv_sb[:]
                    )

            pre = [(jt0, iq0) for jt0 in range(6) for iq0 in range(2)]
            for n, (jt0, iq0) in enumerate(pre):
                s_tile(jt0, iq0)
                if n in (1, 3, 5, 7):
                    v_group((n - 1) // 2)

        # ---- phase 2+3 fused: scores/exp interleaved with y accum ------
        po = tc.alloc_tile_pool(name="ps_o", bufs=2, space="PSUM")
        if True:
            # f32r so the tail's identity matmuls can consume it directly
            y_acc = yp.tile([P, 8, 1024], F32R, tag="y_acc", name="y_acc")

            def out_chain(g, idx, pool=None):
                # one accumulation chain: the group's jts into (iq, ch)
                iq, ch = divmod(idx, 2)
                j0, j1 = GRP[g]
                ops = (pool or po).tile([P, 1024], F32, tag="og", name="og")
                for j2 in range(j0, j1):
                    lhs = v_bf[:, j2, ch * P:(ch + 1) * P]
                    for t in range(2):
                        nc.tensor.matmul(
                            ops[:, t * 512:(t + 1) * 512], lhs,
                            attn[:, j2, iq * 1024 + t * 512: iq * 1024 + (t + 1) * 512],
                            start=(j2 == j0), stop=(j2 == j1 - 1),
                        )
                if g == 0:
                    nc.vector.tensor_copy(y_acc[:, idx, :], ops[:])
                else:
                    nc.vector.tensor_add(y_acc[:, idx, :], ops[:], y_acc[:, idx, :])

            def d_scale(jt):
                # per-jt denominator (sum the 4 chunk sums) + vp scaling
                nc.vector.reduce_sum(
                    dsum_all[:, jt:jt + 1], d_all[:, jt * 4:jt * 4 + 4],
                    axis=mybir.AxisListType.X,
                )
                nc.vector.reciprocal(
                    dsum_all[:, 16 + jt:17 + jt], dsum_all[:, jt:jt + 1]
                )
                nc.vector.tensor_scalar_mul(
                    v_bf[:, jt, :], v_bf[:, jt, :],
                    dsum_all[:, 16 + jt:17 + jt],
                )

            for jt in range(14):
                for iq in range(4):
                    if jt < 6 and iq < 2:
                        continue  # pre-emitted during the projection phase
                    s_tile(jt, iq)
                # spread jt14/15's first chunks early so both D's close
                # right as the exp stream ends, shrinking the tail
                if jt == 12:
                    s_tile(14, 0)
                    s_tile(15, 0)
                elif jt == 13:
                    s_tile(14, 1)
                    s_tile(15, 1)
                d_scale(jt)
                if 6 <= jt <= 9:
                    out_chain(0, 2 * (jt - 6))
                    out_chain(0, 2 * (jt - 6) + 1)
                elif 10 <= jt <= 13:
                    out_chain(1, 2 * (jt - 10))
                    out_chain(1, 2 * (jt - 10) + 1)
                if jt == 13:
                    # group 2 (jts 10-13) is complete: start half its
                    # chains here so the endgame isn't chain-heavy
                    for idx in range(4):
                        out_chain(2, idx)
            # jt14 block: the remaining four exp chunks stream on ACT while
            # the PE runs group 2's second half
            s_tile(14, 2)
            s_tile(15, 2)
            s_tile(14, 3)
            s_tile(15, 3)
            d_scale(14)
            d_scale(15)
            for idx in range(4, 8):
                out_chain(2, idx)
            # final group {14,15}: release the score/chain pools and run the
            # last 8 chains on a 4-deep PSUM rotation. For half the slices
            # the y_acc partial is merged into the chain PSUM with an f32r
            # identity matmul so the drain is a pure copy on the otherwise
            # idle ACT engine; the rest use DVE adds — the two engines
            # drain concurrently, and the y DMAs alternate queues.
            po.release()
            psp.release()
            po4 = tc.alloc_tile_pool(name="ps_o4", bufs=4, space="PSUM")
            idn_r = idn[:]
            for idx in range(8):
                iq, ch = divmod(idx, 2)
                on_act = (idx % 2 == 0)
                ops = po4.tile([P, 1024], F32, tag="og", name="og")
                for t in range(2):
                    sl = slice(t * 512, (t + 1) * 512)
                    isl = slice(iq * 1024 + t * 512, iq * 1024 + (t + 1) * 512)
                    if on_act:
                        nc.tensor.matmul(
                            ops[:, sl], idn_r,
                            y_acc[:, idx, sl],
                            start=True, stop=False,
                        )
                    for j2 in (14, 15):
                        nc.tensor.matmul(
                            ops[:, sl], v_bf[:, j2, ch * P:(ch + 1) * P],
                            attn[:, j2, isl],
                            start=(not on_act and j2 == 14), stop=(j2 == 15),
                        )
                y_sb = ysb_pool.tile([P, 1024], F32, tag="ysb", name="ysb")
                if on_act:
                    nc.scalar.copy(y_sb[:], ops[:])
                else:
                    nc.vector.tensor_add(y_sb[:], ops[:], y_acc[:, idx, :])
                eng = nc.sync if idx % 2 == 0 else nc.gpsimd
                eng.dma_start(
                    y_t.ap()[ch * P:(ch + 1) * P, iq * 1024:(iq + 1) * 1024],
                    y_sb[:],
                )
            po4.release()

_nc_cache = None
LAST_EXEC_TIME_NS = None


def _get_nc():
    global _nc_cache
    if _nc_cache is None:
        _nc_cache = _build_module()
    return _nc_cache


def kernel(x, wq, bq, wk, bk, wv, bv, wp, bp):
    global LAST_EXEC_TIME_NS
    nc = _get_nc()

    import ml_dtypes
    bf = ml_dtypes.bfloat16
    x = np.asarray(x, dtype=np.float32).reshape(B, C, N).astype(bf)
    wq32 = np.asarray(wq, dtype=np.float32)
    wk32 = np.asarray(wk, dtype=np.float32)
    wv32 = np.asarray(wv, dtype=np.float32)
    wp32 = np.asarray(wp, dtype=np.float32)
    a_m = wq32.T @ wk32                   # fold q away: s = x^T (A x + c1) + const_j
    c1 = wq32.T @ np.asarray(bk, dtype=np.float32)
    w2 = wp32 @ wv32                      # fold the output projection into v
    wT = np.ascontiguousarray(np.stack([a_m.T, w2.T])).astype(bf)
    b1 = np.ascontiguousarray(c1.reshape(1, C))
    bv2 = np.ascontiguousarray(
        (wp32 @ np.asarray(bv, dtype=np.float32)).reshape(1, C))
    bp1 = np.asarray(bp, dtype=np.float32).reshape(C)

    ident = np.eye(P, dtype=np.float32)
    in_maps = []
    for core in range(8):
        b, h = divmod(core, 2)
        xb = x[b] if h == 0 else np.ascontiguousarray(np.roll(x[b], -NJ, axis=1))
        in_maps.append({"x": xb, "wT": wT, "b": b1, "bv": bv2, "ident": ident})

    res = bass_utils.run_bass_kernel_spmd(nc, in_maps, core_ids=list(range(8)))
    if res.exec_time_ns is not None:
        LAST_EXEC_TIME_NS = res.exec_time_ns

    y = np.zeros((B, C, N), np.float32)
    for b in range(B):
        y[b] = res.results[2 * b]["y"] + np.roll(res.results[2 * b + 1]["y"], NJ, axis=1)
    y += bp1.reshape(1, C, 1)
    return y.reshape(B, C, 64, 64)


# revision 36
# speedup vs baseline: 1.0506x; 1.0010x over previous
"""Trainium2 Bass kernel for the AttnBlock-style attention module.

Reference computation (note softmax over axis=1, the *i* axis):
    q = wq @ x + bq ; k = wk @ x + bk ; v = wv @ x + bv      (per-pixel 1x1 conv)
    s[b,i,j] = (q[b,:,i] . k[b,:,j]) * C**-0.5
    attn = softmax_i(s)                                      (normalize over i!)
    out[b,c,i] = sum_j attn[b,i,j] v[b,c,j]
    y = wp @ out + bp

Math folding: softmax over i is invariant to per-j constants, so
    s_ij = q_i.k_j = x_i^T (A x_j + c1) + (const_j)    with A = wq^T wk,
                                                       c1 = wq^T bk.
The q projection disappears entirely: the moving side of the score GEMM is
raw x (already in SBUF from the input DMA), and only u = A x + c1 is
projected for the j-columns. Likewise wp is folded into v (w2 = wp@wv).

Sharding: 8 cores = 4 batches x 2 j-halves. The softmax over i is local to a
j-split (it normalizes each attention *column* j over all i). Each core gets x
with its j-half rotated to columns 0..2047 (a pure permutation of the pixel
axis; the host un-rotates the partial output). Each core:
  - computes u (j-half) and v (j-half) with bf16 GEMMs,
  - s_T[j, i] = u^T x  (j on partitions -> softmax reduction is free-axis),
  - attn = exp(s/16) stored unnormalized in bf16; per-j denominators D[j]
    via the fused activation accum_out; 1/D folded into v rows,
  - out_partial[c, i] = sum_{j in half} v_scaled[c,j] attn_T[j,i],
  - y_partial accumulated in f32 SBUF (bias bp added on host).
Host un-rotates and sums the two j-half partials per batch and adds bp.

Out-accumulation is staged in five j-groups (6,4,4,1,1): the chains for a
group are interleaved into later jts' score/exp stream so the PE fills the
ACT-bound stretches, and the tiny trailing groups shrink the post-last-exp
tail to ~4 matmuls per (iq,ch) slice.
"""

import numpy as np

import concourse.bass as bass
import concourse.mybir as mybir
import concourse.tile as tile
from concourse import bacc
from concourse import bass_utils

P = 128
B = 4
C = 256
N = 4096          # 64*64 pixels
NJ = 2048         # j columns per core
NJT = NJ // P     # 16 j tiles
SCALE = 1.0 / np.sqrt(C).item()   # 1/16

F32 = mybir.dt.float32
F32R = mybir.dt.float32r
BF16 = mybir.dt.bfloat16
AF = mybir.ActivationFunctionType

# j-tile boundaries of the out-accumulation groups (last one is inlined)
GRP = [(0, 6), (6, 10), (10, 14), (14, 16)]


def _build_module():
    nc = bacc.Bacc("TRN2", target_bir_lowering=False, debug=False, num_devices=8)

    x_t = nc.dram_tensor("x", [C, N], BF16, kind="ExternalInput")
    w_t = nc.dram_tensor("wT", [2, C, C], BF16, kind="ExternalInput")  # A.T, (wp@wv).T
    b_t = nc.dram_tensor("b", [1, C], F32, kind="ExternalInput")      # c1 = wq.T @ bk
    bv_t = nc.dram_tensor("bv", [1, C], F32, kind="ExternalInput")    # wp@bv
    id_t = nc.dram_tensor("ident", [P, P], BF16, kind="ExternalInput")
    y_t = nc.dram_tensor("y", [C, N], F32, kind="ExternalOutput")

    with tile.TileContext(nc) as tc:
        _emit(nc, tc, x_t, w_t, b_t, bv_t, id_t, y_t)
    nc.compile()
    return nc


def _emit(nc, tc, x_t, w_t, b_t, bv_t, id_t, y_t):
    from contextlib import ExitStack

    with ExitStack() as top:
        const = top.enter_context(tc.tile_pool(name="const", bufs=1))
        big = top.enter_context(tc.tile_pool(name="big", bufs=1))

        # ---- u's weight first on the sync queue, then the x blocks ------
        # w_all[:, 2*w + ci, :] = rows ci*128.. of weight w's transpose
        # slots 0,1: A.T chunks; 2,3: w2.T chunks; 4,5: bv as f32 bits
        w_all = const.tile([P, 6, C], BF16, tag="w_all", name="w_all")
        nc.sync.dma_start(
            w_all[:, 0:2, :].rearrange("p w f -> p (w f)").rearrange(
                "p (c f) -> p c f", c=2),
            bass.AP(tensor=w_t, offset=0, ap=[[C, P], [P * C, 2], [1, C]]),
        )

        xp = top.enter_context(tc.tile_pool(name="xload", bufs=1))
        XBLK = [(0, 512), (512, 512), (1024, 1024), (2048, 1024), (3072, 1024)]
        xb = [xp.tile([P, 2, w], BF16, tag=f"xb{b}", name=f"xb{b}")
              for b, (lo, w) in enumerate(XBLK)]
        for b, (lo, w) in enumerate(XBLK):
            if b < 2:
                # early blocks gate the projections: pure-2D patterns keep
                # the descriptor count (and thus transfer latency) down
                for ci in range(2):
                    nc.sync.dma_start(
                        xb[b][:, ci, :],
                        bass.AP(tensor=x_t, offset=ci * P * N + lo,
                                ap=[[N, P], [1, w]]),
                    )
            else:
                nc.sync.dma_start(
                    xb[b][:],
                    bass.AP(tensor=x_t, offset=lo,
                            ap=[[N, P], [P * N, 2], [1, w]]),
                )

        def xsl(ci, lo, size):
            # x[ci*128:(ci+1)*128, lo:lo+size] as an AP (within one block)
            for b, (blo, w) in enumerate(XBLK):
                if blo <= lo and lo + size <= blo + w:
                    return xb[b][:, ci, lo - blo:lo - blo + size]
            raise AssertionError((lo, size))

        # ---- remaining constants on the gpsimd DMA queue ---------------
        nc.gpsimd.dma_start(
            w_all[:, 2:4, :].rearrange("p w f -> p (w f)").rearrange(
                "p (c f) -> p c f", c=2),
            bass.AP(tensor=w_t, offset=C * C, ap=[[C, P], [P * C, 2], [1, C]]),
        )

        # b_all columns 0,1 = c1 halves
        b_all = const.tile([P, 2], F32, tag="b_all", name="b_all")
        nc.gpsimd.dma_start(
            b_all[:], bass.AP(tensor=b_t, offset=0, ap=[[1, P], [P, 2]]),
        )
        bv_sb = w_all[:, 4:6, :].rearrange("p a b -> p (a b)").bitcast(F32)
        nc.gpsimd.dma_start(
            bv_sb[:], bass.AP(tensor=bv_t, offset=0, ap=[[0, P], [1, C]])
        )
        # f32 identity for the tail's y_acc-into-PSUM merge matmuls
        idn = const.tile([P, P], BF16, tag="idn", name="idn")
        nc.gpsimd.dma_start(
            idn[:], bass.AP(tensor=id_t, offset=0, ap=[[P, P], [1, P]])
        )

        # ---- persistent activations -----------------------------------
        yp = top.enter_context(tc.tile_pool(name="yaccp", bufs=1))
        ysb_pool = top.enter_context(tc.tile_pool(name="ysb", bufs=4))
        # u laid out [p, ci, j]: dim1 = the two c-halves (score contraction)
        u_bf = big.tile([P, 2, NJ], BF16, tag="u_bf", name="u_bf")
        v_bf = big.tile([P, NJT, C], BF16, tag="v_bf", name="v_bf")
        # attn [p, jt, i]
        attn = big.tile([P, NJT, N], BF16, tag="attn", name="attn")
        # d_all cols 0:64 = per-(jt,iq) exp sums, 64:80 = D, 80:96 = 1/D
        d_all = big.tile([P, 96], F32, tag="d_all", name="d_all")
        dsum_all = d_all[:, 64:96]

        # ---- warmups: run while the x DMA streams in -------------------
        # dummy matmuls lift the PE HAM clock-gate to 8/8 before real work
        # arrives; a dummy Exp pulls the ~2.7us ACT table load off the
        # critical path of the first score tile.
        with tc.tile_pool(name="warm", bufs=1) as wp_pool, \
             tc.tile_pool(name="warm_ps", bufs=1, space="PSUM") as wpp:
            wsb = wp_pool.tile([P, 512], BF16, tag="wsb", name="wsb")
            wex = wsb[:, 508:509]
            wps = wpp.tile([P, 512], F32, tag="wps", name="wps")
            nc.vector.memset(wsb[:], 0.0)
            # small warms: ramp the PE clock without hogging the queue
            # (the real work is gated on the x/w DMAs anyway)
            for _ in range(6):
                nc.tensor.matmul(wps[:, 0:P], wsb[:, 0:P], wsb[:, 0:P],
                                 start=True, stop=True)
            nc.scalar.activation(wex[:], wps[:, 0:1], AF.Exp, scale=0.0)

        psp = tc.alloc_tile_pool(name="ps_s", bufs=2, space="PSUM")

        def s_tile(jt, iq):
            # one [128,1024] score tile + exp(+accum) into the attn store
            ps = psp.tile([P, 1024], F32, tag="s", name="s_ps")
            for ci in range(2):
                lhs = u_bf[:, ci, jt * P:(jt + 1) * P]
                for t in range(2):
                    nc.tensor.matmul(
                        ps[:, t * 512:(t + 1) * 512], lhs,
                        xsl(ci, iq * 1024 + t * 512, 512),
                        start=(ci == 0), stop=(ci == 1),
                    )
            nc.scalar.activation(
                attn[:, jt, iq * 1024:(iq + 1) * 1024], ps[:],
                AF.Exp, scale=float(SCALE),
                accum_out=d_all[:, jt * 4 + iq: jt * 4 + iq + 1],
            )

        # ---- phase 1: u projection, early score tiles, v projection ----
        # u first (it gates the scores), then 12 pre-emitted score tiles
        # (they only need x blocks 0-2) so ACT's exp stream starts early,
        # then the v projections (not needed until the first 1/D scaling).
        with tc.tile_pool(name="ps_qkv", bufs=4, space="PSUM") as pq:
            for blk in range(2):
                for ch in range(2):
                    pss = [pq.tile([P, 512], F32, tag="ps", name="ps") for _ in range(2)]
                    for ci in range(2):
                        lhs = w_all[:, ci, ch * P:(ch + 1) * P]
                        for t2 in range(2):
                            t = blk * 2 + t2
                            nc.tensor.matmul(
                                pss[t2][:], lhs,
                                xsl(ci, t * 512, 512),
                                start=(ci == 0), stop=(ci == 1),
                            )
                    for t2 in range(2):
                        t = blk * 2 + t2
                        nc.vector.tensor_scalar_add(
                            u_bf[:, ch, t * 512:(t + 1) * 512], pss[t2][:],
                            b_all[:, ch:ch + 1])

            # pre-emitted score tiles interleaved with the v projections:
            # the in-order PE queue would otherwise stall on the score
            # PSUM rotation (ACT paces the exps) with the v work stuck
            # behind it.
            def v_group(jtg):
                pss = [pq.tile([P, C], F32, tag="ps", name="ps") for _ in range(4)]
                for ci in range(2):
                    for t in range(4):
                        jt = jtg * 4 + t
                        nc.tensor.matmul(
                            pss[t][:],
                            xsl(ci, jt * P, P),
                            w_all[:, 2 + ci, :],
                            start=(ci == 0), stop=(ci == 1),
                        )
                for t in range(4):
                    nc.vector.tensor_add(
                        v_bf[:, jtg * 4 + t, :], pss[t][:], bv_sb[:]
                    )

            pre = [(jt0, iq0) for jt0 in range(6) for iq0 in range(2)]
            for n, (jt0, iq0) in enumerate(pre):
                s_tile(jt0, iq0)
                if n in (1, 3, 5, 7):
                    v_group((n - 1) // 2)

        # ---- phase 2+3 fused: scores/exp interleaved with y accum ------
        po = tc.alloc_tile_pool(name="ps_o", bufs=2, space="PSUM")
        if True:
            # bf16 so the tail's identity-merge matmuls run at full rate
            # (f32r lowers to slow fp32_mode=HIGH matmuls)
            y_acc = yp.tile([P, 8, 1024], BF16, tag="y_acc", name="y_acc")

            def out_chain(g, idx, pool=None):
                # one accumulation chain: the group's jts into (iq, ch)
                iq, ch = divmod(idx, 2)
                j0, j1 = GRP[g]
                ops = (pool or po).tile([P, 1024], F32, tag="og", name="og")
                for j2 in range(j0, j1):
                    lhs = v_bf[:, j2, ch * P:(ch + 1) * P]
                    for t in range(2):
                        nc.tensor.matmul(
                            ops[:, t * 512:(t + 1) * 512], lhs,
                            attn[:, j2, iq * 1024 + t * 512: iq * 1024 + (t + 1) * 512],
                            start=(j2 == j0), stop=(j2 == j1 - 1),
                        )
                if g == 0:
                    nc.vector.tensor_copy(y_acc[:, idx, :], ops[:])
                else:
                    nc.vector.tensor_add(y_acc[:, idx, :], ops[:], y_acc[:, idx, :])

            def d_scale(jt):
                # per-jt denominator (sum the 4 chunk sums) + vp scaling
                nc.vector.reduce_sum(
                    dsum_all[:, jt:jt + 1], d_all[:, jt * 4:jt * 4 + 4],
                    axis=mybir.AxisListType.X,
                )
                nc.vector.reciprocal(
                    dsum_all[:, 16 + jt:17 + jt], dsum_all[:, jt:jt + 1]
                )
                nc.vector.tensor_scalar_mul(
                    v_bf[:, jt, :], v_bf[:, jt, :],
                    dsum_all[:, 16 + jt:17 + jt],
                )

            for jt in range(14):
                for iq in range(4):
                    if jt < 6 and iq < 2:
                        continue  # pre-emitted during the projection phase
                    s_tile(jt, iq)
                # spread jt14/15's first chunks early so both D's close
                # right as the exp stream ends, shrinking the tail
                if jt == 12:
                    s_tile(14, 0)
                    s_tile(15, 0)
                elif jt == 13:
                    s_tile(14, 1)
                    s_tile(15, 1)
                d_scale(jt)
                if 6 <= jt <= 9:
                    out_chain(0, 2 * (jt - 6))
                    out_chain(0, 2 * (jt - 6) + 1)
                elif 10 <= jt <= 13:
                    out_chain(1, 2 * (jt - 10))
                    out_chain(1, 2 * (jt - 10) + 1)
                if jt == 13:
                    # group 2 (jts 10-13) is complete: start half its
                    # chains here so the endgame isn't chain-heavy
                    for idx in range(4):
                        out_chain(2, idx)
            # jt14 block: the remaining four exp chunks stream on ACT while
            # the PE runs group 2's second half
            s_tile(14, 2)
            s_tile(15, 2)
            s_tile(14, 3)
            s_tile(15, 3)
            d_scale(14)
            d_scale(15)
            for idx in range(4, 8):
                out_chain(2, idx)
            # final group {14,15}: release the score/chain pools and run the
            # last 8 chains on a 4-deep PSUM rotation. For half the slices
            # the y_acc partial is merged into the chain PSUM with an f32r
            # identity matmul so the drain is a pure copy on the otherwise
            # idle ACT engine; the rest use DVE adds — the two engines
            # drain concurrently, and the y DMAs alternate queues.
            po.release()
            psp.release()
            po4 = tc.alloc_tile_pool(name="ps_o4", bufs=4, space="PSUM")
            idn_r = idn[:]
            for idx in range(8):
                iq, ch = divmod(idx, 2)
                on_act = idx not in (3, 7)
                ops = po4.tile([P, 1024], F32, tag="og", name="og")
                for t in range(2):
                    sl = slice(t * 512, (t + 1) * 512)
                    isl = slice(iq * 1024 + t * 512, iq * 1024 + (t + 1) * 512)
                    nc.tensor.matmul(
                        ops[:, sl], idn_r, y_acc[:, idx, sl],
                        start=True, stop=False,
                    )
                    for j2 in (14, 15):
                        nc.tensor.matmul(
                            ops[:, sl], v_bf[:, j2, ch * P:(ch + 1) * P],
                            attn[:, j2, isl],
                            start=False, stop=(j2 == 15),
                        )
                y_sb = ysb_pool.tile([P, 1024], F32, tag="ysb", name="ysb")
                if on_act:
                    nc.scalar.copy(y_sb[:], ops[:])
                else:
                    nc.vector.tensor_copy(y_sb[:], ops[:])
                eng = nc.sync if idx % 2 == 0 else nc.gpsimd
                eng.dma_start(
                    y_t.ap()[ch * P:(ch + 1) * P, iq * 1024:(iq + 1) * 1024],
                    y_sb[:],
                )
            po4.release()

_nc_cache = None
LAST_EXEC_TIME_NS = None


def _get_nc():
    global _nc_cache
    if _nc_cache is None:
        _nc_cache = _build_module()
    return _nc_cache


def kernel(x, wq, bq, wk, bk, wv, bv, wp, bp):
    global LAST_EXEC_TIME_NS
    nc = _get_nc()

    import ml_dtypes
    bf = ml_dtypes.bfloat16
    x = np.asarray(x, dtype=np.float32).reshape(B, C, N).astype(bf)
    wq32 = np.asarray(wq, dtype=np.float32)
    wk32 = np.asarray(wk, dtype=np.float32)
    wv32 = np.asarray(wv, dtype=np.float32)
    wp32 = np.asarray(wp, dtype=np.float32)
    a_m = wq32.T @ wk32                   # fold q away: s = x^T (A x + c1) + const_j
    c1 = wq32.T @ np.asarray(bk, dtype=np.float32)
    w2 = wp32 @ wv32                      # fold the output projection into v
    wT = np.ascontiguousarray(np.stack([a_m.T, w2.T])).astype(bf)
    b1 = np.ascontiguousarray(c1.reshape(1, C))
    bv2 = np.ascontiguousarray(
        (wp32 @ np.asarray(bv, dtype=np.float32)).reshape(1, C))
    bp1 = np.asarray(bp, dtype=np.float32).reshape(C)

    ident = np.eye(P, dtype=np.float32).astype(ml_dtypes.bfloat16)
    in_maps = []
    for core in range(8):
        b, h = divmod(core, 2)
        xb = x[b] if h == 0 else np.ascontiguousarray(np.roll(x[b], -NJ, axis=1))
        in_maps.append({"x": xb, "wT": wT, "b": b1, "bv": bv2, "ident": ident})

    res = bass_utils.run_bass_kernel_spmd(nc, in_maps, core_ids=list(range(8)))
    if res.exec_time_ns is not None:
        LAST_EXEC_TIME_NS = res.exec_time_ns

    y = np.zeros((B, C, N), np.float32)
    for b in range(B):
        y[b] = res.results[2 * b]["y"] + np.roll(res.results[2 * b + 1]["y"], NJ, axis=1)
    y += bp1.reshape(1, C, 1)
    return y.reshape(B, C, 64, 64)
